# revision 38
# baseline (speedup 1.0000x reference)
"""Self-contained Trainium2 Bass kernel for a single attention head.

Reference computation (per batch b):
    Q = x @ Wq + bq ; K = x @ Wk + bk ; V = x @ Wv + bv      (x: [S, M])
    out = softmax(Q K^T / sqrt(D)) @ V                        ([S, D])

Shapes: B=4, S=4096, M=1024, D=128, f32.

Sharding: 8 cores; core c handles batch b=c//2, query-half h=c%2 (2048 query
rows), with the full batch (4096 rows) as keys/values. Softmax is over the
key axis only, so key order is irrelevant: the host permutes each core's
batch so its own query rows come first, and pre-transposes to xT [M, S] so
the device needs no input transposes. No collectives.

Device layout (per core):
  - projections contract over M with fp32r matmuls: Q^T, K^T produced
    dk-major [128, s]; V produced naturally [s, 128] via 128x128 transposes.
  - scores computed transposed: S^T[s, q] = (K^T tile).T @ Q^T, fp32r,
    moving dim 512. exp (ACT engine) writes A^T bf16 - which is exactly the
    layout attn@V needs, so no O(S*S) transposes.
  - softmax denominator: level-1 pairwise bf16 adds on DVE (fast 2-byte
    mode), f32 wide accumulation -> [128, q] partials, one tiny f32
    ones-matmul -> [1, q], transpose + reciprocal -> per-q-row scale applied
    after the final O^T -> O transpose.
  - attn@V: O^T[dv, q] accumulated in PSUM over 32 bf16 matmuls.
  - fp32r trick: fp32r-typed DRAM inputs may feed fp32r matmuls directly
    (the BIR verifier accepts ExternalInput as pre-rounded), so x needs no
    on-device rounding pass. fp32r matmuls run at full PE rate for moving
    dims >= 256 with ~1.5e-4 error.
  - engine placement: PE matmuls ~113us (bottleneck), ACT exp+biases+psum
    copies ~94us, DVE den+normalize ~63us (cost-model estimates per core).
    GPSIMD must not touch PSUM (walrus restriction).
"""

from contextlib import ExitStack

import numpy as np

import concourse.bass as bass
import concourse.tile as tile
from concourse import bacc, mybir
from concourse.bass_utils import run_bass_kernel_spmd
from concourse.masks import make_identity

F32 = mybir.dt.float32
F32R = mybir.dt.float32r
BF16 = mybir.dt.bfloat16

B, S, M, D = 4, 4096, 1024, 128
N_CORES = 8
SCALE = 1.0 / np.sqrt(np.float32(D))


def build_attention(nc, S_keys=S, S_q=S // 2, M_dim=M, SC=512, QC=512,
                    repeat=1, phases=(1, 2), pair=False):
    """Emit the attention graph. S_keys: key rows; S_q: query rows (prefix of
    the permuted sequence); SC: phase-1 s-chunk; QC: phase-2 q-chunk.
    repeat>1 re-emits the whole body (for dispatch-free timing)."""
    P = 128
    MT = M_dim // P              # m-tiles
    ST = S_keys // P             # key s-tiles
    S_own = S_q if pair else S_keys   # key rows this core projects
    ST_own = S_own // P
    NSC = S_own // SC            # phase-1 chunks
    NSCQ = S_q // SC             # phase-1 chunks that also need Q
    NQC = S_q // QC              # phase-2 q-chunks
    SCT = SC // P                # 128-tiles per s-chunk
    QT = QC // P                 # 128-tiles per q-chunk
    GB = S_keys // S_own         # gather slots (2 in pair mode, else 1)
    assert not pair or GB == 2

    xT = nc.dram_tensor("xT", [M_dim, S_own], F32R, kind="ExternalInput").ap()
    wq = nc.dram_tensor("wq", [M_dim, D], F32R, kind="ExternalInput").ap()
    wk = nc.dram_tensor("wk", [M_dim, D], F32R, kind="ExternalInput").ap()
    wv = nc.dram_tensor("wv", [M_dim, D], F32R, kind="ExternalInput").ap()
    bq = nc.dram_tensor("bq", [D, 1], F32, kind="ExternalInput").ap()
    bk = nc.dram_tensor("bk", [D, 1], F32, kind="ExternalInput").ap()
    bv = nc.dram_tensor("bv", [D, 1], F32, kind="ExternalInput").ap()
    out = nc.dram_tensor("out", [S_q, D], F32, kind="ExternalOutput").ap()

    xT_r = xT.rearrange("(t p) s -> p t s", p=P)
    out_r = out.rearrange("(t p) d -> p t d", p=P)

    with tile.TileContext(nc) as tc:
      for _rep in range(repeat):
        ctx = ExitStack()
        persist = ctx.enter_context(tc.tile_pool(name="persist", bufs=1))

        ident = persist.tile([P, P], F32)
        make_identity(nc, ident[:])
        ident_r = persist.tile([P, P], F32R)
        nc.vector.tensor_copy(ident_r[:], ident[:])
        ones_col = persist.tile([P, 1], F32)
        nc.vector.memset(ones_col[:], 1.0)

        # weights/biases: fp32r-typed DRAM params DMA directly to fp32r tiles
        w_r = []
        b_sb = []
        for name, w_ap, b_ap in (("k", wk, bk), ("q", wq, bq), ("v", wv, bv)):
            wr = persist.tile([P, MT, D], F32R, name=f"w{name}_r")
            nc.scalar.dma_start(wr[:], w_ap.rearrange("(t p) d -> p t d", p=P))
            w_r.append(wr)
            bs = persist.tile([P, 1], F32, name=f"b{name}_sb")
            nc.scalar.dma_start(bs[:], b_ap)
            b_sb.append(bs)
        wk_r, wq_r, wv_r = w_r
        bk_sb, bq_sb, bv_sb = b_sb

        kT_sb = persist.tile([P, GB, S_own], F32R)  # K^T  [dk, slot, s]
        qT_sb = persist.tile([P, S_q], F32R)      # Q^T  [dk, q]
        v_sb = persist.tile([P, ST, D], BF16)     # V    [s%128, s-tile, dv]
        o_sb = persist.tile([P, S_q // P, D], F32)  # O   [q%128, q-tile, dv]

        def kt_tile(st):
            return kT_sb[:, st // ST_own, bass.ts(st % ST_own, P)]

        Ident = mybir.ActivationFunctionType.Identity
        Exp = mybir.ActivationFunctionType.Exp
        Copy = mybir.ActivationFunctionType.Copy

        # ---- phase 1: projections ----
        if 1 in phases:
          with (
            tc.tile_pool(name="xstage", bufs=3) as xstage,
            tc.tile_pool(name="vtmp", bufs=2) as vtmp,
            tc.tile_pool(name="dram", bufs=1, space="DRAM") as drampool,
            tc.tile_pool(name="p1psum", bufs=2, space="PSUM") as p1psum,
            tc.tile_pool(name="p1tpsum", bufs=2, space="PSUM") as p1tpsum,
        ):
            if pair:
                # own-half staging; gathered via pair AllGather below
                kT_own = persist.tile([P, S_own], F32R)
                v_own = persist.tile([P, ST_own, D], BF16)
            else:
                kT_own = kT_sb[:, 0, :]
                v_own = v_sb

            for sc in range(NSC):
                ssl = bass.ds(sc * SC, SC)
                x_r = xstage.tile([P, MT, SC], F32R)
                (nc.sync if sc % 2 == 0 else nc.scalar).dma_start(
                    x_r[:], xT_r[:, :, ssl])

                # K^T chunk
                ps_k = p1psum.tile([P, SC], F32)
                for mt in range(MT):
                    nc.tensor.matmul(ps_k[:], wk_r[:, mt, :], x_r[:, mt, :],
                                     start=(mt == 0), stop=(mt == MT - 1))
                nc.scalar.activation(kT_own[:, ssl], ps_k[:], Ident,
                                     bias=bk_sb[:])

                # Q^T chunk (query rows are the permuted prefix)
                if sc < NSCQ:
                    ps_q = p1psum.tile([P, SC], F32)
                    for mt in range(MT):
                        nc.tensor.matmul(ps_q[:], wq_r[:, mt, :], x_r[:, mt, :],
                                         start=(mt == 0), stop=(mt == MT - 1))
                    nc.scalar.activation(qT_sb[:, ssl], ps_q[:], Ident,
                                         bias=bq_sb[:])

                # V^T chunk, then transpose to natural V tiles
                ps_v = p1psum.tile([P, SC], F32)
                for mt in range(MT):
                    nc.tensor.matmul(ps_v[:], wv_r[:, mt, :], x_r[:, mt, :],
                                     start=(mt == 0), stop=(mt == MT - 1))
                vt = vtmp.tile([P, SC], F32R)
                nc.scalar.activation(vt[:], ps_v[:], Ident, bias=bv_sb[:])
                for t in range(SCT):
                    ps_t = p1tpsum.tile([P, D], F32R)
                    nc.tensor.transpose(ps_t[:], vt[:, bass.ts(t, P)], ident_r[:])
                    nc.scalar.copy(v_own[:, sc * SCT + t, :], ps_t[:])

            if pair:
                groups = [[i, i + 1] for i in range(0, nc.num_devices, 2)]
                # K^T pair AllGather: [P, S_own] -> [2, P, S_own]
                kb = drampool.tile([P, S_own], F32R)
                kg = drampool.tile([GB * P, S_own], F32R)
                nc.sync.dma_start(kb[:], kT_own[:])
                nc.gpsimd.collective_compute(
                    "AllGather", mybir.AluOpType.bypass,
                    replica_groups=groups, ins=[kb.opt()], outs=[kg.opt()])
                nc.sync.dma_start(
                    kT_sb[:], kg.rearrange("(g p) s -> p g s", p=P))
                # V pair AllGather: [S_own, D] -> [2*S_own, D]
                vb = drampool.tile([S_own, D], BF16)
                vg = drampool.tile([GB * S_own, D], BF16)
                nc.sync.dma_start(vb.rearrange("(t p) d -> p t d", p=P), v_own[:])
                nc.gpsimd.collective_compute(
                    "AllGather", mybir.AluOpType.bypass,
                    replica_groups=groups, ins=[vb.opt()], outs=[vg.opt()])
                nc.sync.dma_start(
                    v_sb[:], vg.rearrange("(t p) d -> p t d", p=P))

        # ---- phase 2: attention ----
        if 2 in phases:
          with (
            tc.tile_pool(name="a_sb", bufs=2) as apool,
            tc.tile_pool(name="dacc", bufs=2) as dpool,
            tc.tile_pool(name="small", bufs=2 * QT) as small,
            tc.tile_pool(name="otmp", bufs=2) as otpool,
            tc.tile_pool(name="spsum", bufs=2, space="PSUM") as spsum,
            tc.tile_pool(name="opsum", bufs=1, space="PSUM") as opsum,
            tc.tile_pool(name="dpsum", bufs=1, space="PSUM") as dpsum,
            tc.tile_pool(name="otpsum", bufs=1, space="PSUM") as otpsum,
        ):
            for qc in range(NQC):
                qsl = bass.ds(qc * QC, QC)
                a_sb = apool.tile([P, ST, QC], BF16)
                # wide denominator accumulator: 4 q-chunk-wide lanes summed at
                # the end (fewer, larger DVE adds)
                DW = 4
                den4 = dpool.tile([P, DW, QC], F32)
                den = dpool.tile([P, QC], F32)

                # pass 1: scores (pairs) + one wide exp per pair
                assert ST % 2 == 0
                for sp in range(ST // 2):
                    ps_s = spsum.tile([P, 2, QC], F32)
                    for j in range(2):
                        nc.tensor.matmul(ps_s[:, j, :],
                                         kt_tile(2 * sp + j),
                                         qT_sb[:, qsl], start=True, stop=True)
                    nc.scalar.activation(a_sb[:, 2 * sp:2 * sp + 2, :], ps_s[:],
                                         Exp, scale=float(SCALE))
                # denominator partials: level-1 pairwise bf16 adds (4x DVE mode),
                # then f32 wide accumulation of the 16 pair-sums
                assert ST % 2 == 0
                npair = ST // 2
                apair = dpool.tile([P, npair, QC], BF16)
                for pr in range(npair):
                    nc.vector.tensor_add(apair[:, pr, :], a_sb[:, 2 * pr, :],
                                         a_sb[:, 2 * pr + 1, :])
                if npair % DW == 0:
                    for g in range(npair // DW):
                        grp = apair[:, g * DW:(g + 1) * DW, :]
                        if g == 0:
                            nc.vector.tensor_copy(den4[:], grp)
                        else:
                            nc.vector.tensor_add(den4[:], den4[:], grp)
                    nc.vector.tensor_add(den4[:, 0, :], den4[:, 0, :], den4[:, 1, :])
                    nc.vector.tensor_add(den4[:, 2, :], den4[:, 2, :], den4[:, 3, :])
                    nc.vector.tensor_add(den[:], den4[:, 0, :], den4[:, 2, :])
                else:
                    nc.vector.tensor_copy(den4[:, :npair, :], apair[:])
                    for pr in range(1, npair):
                        nc.vector.tensor_add(den4[:, 0, :], den4[:, 0, :],
                                             den4[:, pr, :])
                    nc.vector.tensor_copy(den[:], den4[:, 0, :])

                # pass 2: O^T accumulation
                ps_o = opsum.tile([P, QC], F32)
                for st in range(ST):
                    nc.tensor.matmul(ps_o[:], v_sb[:, st, :], a_sb[:, st, :],
                                     start=(st == 0), stop=(st == ST - 1))
                oT = otpool.tile([P, QC], F32)
                nc.scalar.copy(oT[:], ps_o[:])

                # denominator: [128, QC] -> [1, QC] -> transpose -> reciprocal
                ps_d = dpsum.tile([1, QC], F32)
                nc.tensor.matmul(ps_d[:], ones_col[:], den[:], start=True, stop=True)
                den_flat = small.tile([1, QC], F32)
                nc.scalar.copy(den_flat[:], ps_d[:])

                for t in range(QT):
                    ps_dt = dpsum.tile([P, 1], F32)
                    nc.tensor.transpose(ps_dt[:], den_flat[:1, bass.ts(t, P)],
                                        ident[:1, :1])
                    rden = small.tile([P, 1], F32)
                    nc.vector.reciprocal(rden[:], ps_dt[:])
                    ps_ot = otpsum.tile([P, D], F32)
                    nc.tensor.transpose(ps_ot[:], oT[:, bass.ts(t, P)], ident[:])
                    nc.vector.tensor_scalar_mul(o_sb[:, qc * QT + t, :],
                                                ps_ot[:], rden[:])
                (nc.sync if qc % 2 == 0 else nc.scalar).dma_start(
                    out_r[:, qc * QT:(qc + 1) * QT, :],
                    o_sb[:, qc * QT:(qc + 1) * QT, :])
        ctx.close()

    return nc


def build(n_cores=N_CORES, **kw):
    nc = bacc.Bacc("TRN2", target_bir_lowering=False, debug=False,
                   num_devices=n_cores)
    build_attention(nc, **kw)
    nc.compile()
    return nc


PAIR = False


def shard_inputs(input, Wq, bq, Wk, bk, Wv, bv, pair=PAIR):
    """Build per-core in_maps. Core c: batch c//2, query-half c%2. In pair
    mode each core only gets its own half (K/V gathered on-device); otherwise
    it gets the whole batch with its query rows permuted to the front
    (softmax is key-permutation invariant)."""
    half = S // 2
    in_maps = []
    for c in range(N_CORES):
        b, h = divmod(c, 2)
        xb = np.asarray(input[b])
        if pair:
            x_perm = xb[h * half:(h + 1) * half]
        else:
            x_perm = np.concatenate(
                [xb[h * half:(h + 1) * half], xb[(1 - h) * half:(2 - h) * half]],
                axis=0)
        in_maps.append({
            "xT": np.ascontiguousarray(x_perm.T, dtype=np.float32),
            "wq": np.asarray(Wq, dtype=np.float32),
            "wk": np.asarray(Wk, dtype=np.float32),
            "wv": np.asarray(Wv, dtype=np.float32),
            "bq": np.asarray(bq, dtype=np.float32).reshape(D, 1),
            "bk": np.asarray(bk, dtype=np.float32).reshape(D, 1),
            "bv": np.asarray(bv, dtype=np.float32).reshape(D, 1),
        })
    return in_maps


_NC_CACHE = {}


def kernel(input, Wq, bq, Wk, bk, Wv, bv):
    in_maps = shard_inputs(input, Wq, bq, Wk, bk, Wv, bv)
    if "nc" not in _NC_CACHE:
        _NC_CACHE["nc"] = build(pair=PAIR)
    nc = _NC_CACHE["nc"]
    res = run_bass_kernel_spmd(nc, in_maps, core_ids=list(range(N_CORES)))
    half = S // 2
    result = np.empty((B, S, D), dtype=np.float32)
    for c in range(N_CORES):
        b, h = divmod(c, 2)
        result[b, h * half:(h + 1) * half] = res.results[c]["out"]
    return result


if __name__ == "__main__":
    rng = np.random.default_rng(0)
    inputs = {
        "input": rng.standard_normal((B, S, M), dtype=np.float32),
        "Wq": (rng.standard_normal((M, D), dtype=np.float32) / np.sqrt(M)).astype(np.float32),
        "bq": (rng.standard_normal(D, dtype=np.float32) * 0.02),
        "Wk": (rng.standard_normal((M, D), dtype=np.float32) / np.sqrt(M)).astype(np.float32),
        "bk": (rng.standard_normal(D, dtype=np.float32) * 0.02),
        "Wv": (rng.standard_normal((M, D), dtype=np.float32) / np.sqrt(M)).astype(np.float32),
        "bv": (rng.standard_normal(D, dtype=np.float32) * 0.02),
    }
    out = kernel(**inputs)
    print("kernel output:", out.shape, out.dtype)

